# revision 3
# baseline (speedup 1.0000x reference)
"""GAT layer (single head) on Trainium2, 8 NeuronCores.

Strategy (destination-sharded):
  Launch A: per-core dense matmul producing table rows
            [h (48) | a_src | a_dst | zeros] = x @ [W | W@att_src | W@att_dst | 0].
  Host:     sort nodes by (degree, bank0-count) into 784 tiles of 128; pad each
            tile's incident edges into per-(tile,bank) slot rectangles; emit
            biased int16 source-row indices (dma_gather addresses a 65536-row
            window around a biased base pointer, so 100k rows need 2 banks).
  Launch B: per-core: dma_gather edge rows (striped over 4 SWDGE queues),
            edge softmax without max-subtraction (logits bounded), weighted
            aggregation via strided multiply+reduce, ELU, final linear,
            log_softmax.
"""
import numpy as np

N_NODES = 100_000
N_EDGES = 1_600_000
IN_CH = 128
HIDDEN = 48
OUT_CH = 16
NEG_SLOPE = 0.2

P = 128
ROW = 64                      # table row width (f32) -> 256B, dma_gather granule
CORES = 8
NODES_PER_CORE = 12544        # 98 tiles x 128
NT = 98                       # tiles per core
N_TILES = NT * CORES          # 784
N_RANKS = N_TILES * P         # 100352
TABLE_ROWS = 163840           # declared table rows (bank1 window slack)
BANK0_BASE = 32768            # biased base row for bank0 (rows [0, 65536))
BANK1_BASE = 98304            # biased base row for bank1 (rows [65536, 131072))
PAD0_TROW = 65535             # pad row inside bank0 window (biased +32767)
PAD1_TROW = 100001            # pad row inside bank1 window (biased +1697)
PAD_ASRC = -100.0             # a_src of pad rows -> exp(leaky(-100+a)) ~ 0
BATCH = 4                     # tiles per gather call pair
NQ = 4                        # SWDGE queues

_f32 = np.float32


def _build_phase_a():
    import concourse.bacc as bacc
    import concourse.mybir as mybir
    import concourse.tile as tile
    from concourse.masks import make_identity

    nc = bacc.Bacc("TRN2", target_bir_lowering=False, debug=False,
                   num_devices=CORES)
    xT = nc.dram_tensor("xT", [P, NODES_PER_CORE], mybir.dt.float32,
                        kind="ExternalInput")
    W = nc.dram_tensor("W", [IN_CH, HIDDEN], mybir.dt.float32,
                       kind="ExternalInput")
    att = nc.dram_tensor("att", [HIDDEN, 2], mybir.dt.float32,
                         kind="ExternalInput")
    hx = nc.dram_tensor("hx", [P, NT, ROW], mybir.dt.float32,
                        kind="ExternalOutput")

    with tile.TileContext(nc) as tc:
        with (
            tc.tile_pool(name="const", bufs=1) as cp,
            tc.tile_pool(name="xp", bufs=3) as xp,
            tc.tile_pool(name="st", bufs=1) as st,
            tc.tile_pool(name="ps", bufs=2, space="PSUM") as ps,
            tc.tile_pool(name="ps2", bufs=2, space="PSUM") as ps2,
        ):
            ident = cp.tile([P, P], mybir.dt.float32)
            make_identity(nc, ident[:])
            w_sb = cp.tile([IN_CH, HIDDEN], mybir.dt.float32)
            nc.sync.dma_start(out=w_sb[:], in_=W[:, :])
            att_sb = cp.tile([HIDDEN, 2], mybir.dt.float32)
            nc.sync.dma_start(out=att_sb[:], in_=att[:, :])

            # W^T via PE transpose, then Wa = (W^T)^T @ att = W @ att
            wT_ps = ps.tile([HIDDEN, IN_CH], mybir.dt.float32, space="PSUM")
            nc.tensor.transpose(out=wT_ps[:], in_=w_sb[:], identity=ident[:])
            wT_sb = cp.tile([HIDDEN, IN_CH], mybir.dt.float32)
            nc.vector.tensor_copy(out=wT_sb[:], in_=wT_ps[:])
            wa_ps = ps2.tile([P, 2], mybir.dt.float32, space="PSUM")
            nc.tensor.matmul(out=wa_ps[:], lhsT=wT_sb[:], rhs=att_sb[:],
                             start=True, stop=True)

            rhs_all = cp.tile([IN_CH, ROW], mybir.dt.float32)
            nc.vector.memset(rhs_all[:], 0.0)
            nc.vector.tensor_copy(out=rhs_all[:, 0:HIDDEN], in_=w_sb[:])
            nc.vector.tensor_copy(out=rhs_all[:, HIDDEN:HIDDEN + 2],
                                  in_=wa_ps[:])

            stage = st.tile([P, NT, ROW], mybir.dt.float32)
            for t in range(NT):
                xt = xp.tile([P, P], mybir.dt.float32, tag="xt")
                nc.sync.dma_start(out=xt[:], in_=xT[:, t * P:(t + 1) * P])
                h_ps = ps.tile([P, ROW], mybir.dt.float32, space="PSUM",
                               tag="hps")
                nc.tensor.matmul(out=h_ps[:], lhsT=xt[:], rhs=rhs_all[:],
                                 start=True, stop=True)
                nc.vector.tensor_copy(out=stage[:, t, :], in_=h_ps[:])
            nc.sync.dma_start(out=hx[:, :, :], in_=stage[:])

    nc.finalize()
    return nc


def _layout(edge_src, edge_dst):
    """Build the padded 2-bank tile layout. Returns schedule + per-core arrays."""
    E = edge_src.shape[0]
    trow = edge_src + (edge_src >= PAD0_TROW)          # table row of source
    bank = (trow >= 65536).astype(np.int8)

    deg = np.bincount(edge_dst, minlength=N_NODES)
    n1 = np.bincount(edge_dst[bank == 1], minlength=N_NODES)
    n0 = deg - n1

    # node order: group similar (deg, n0) so per-tile bank rectangles are tight
    order = np.lexsort((-n0, -deg))                    # primary -deg, then -n0
    node_at_rank = np.full(N_RANKS, -1, np.int64)
    node_at_rank[:N_NODES] = order
    rank_of_node = np.empty(N_NODES, np.int64)
    rank_of_node[order] = np.arange(N_NODES)

    r = rank_of_node[edge_dst]
    tile_g = r >> 7                                    # global tile 0..783
    p = r & 127

    # per-global-tile per-bank max counts
    n0r = np.zeros(N_RANKS, np.int64)
    n0r[:N_NODES] = n0[order]
    n1r = np.zeros(N_RANKS, np.int64)
    n1r[:N_NODES] = n1[order]
    d0_tile = n0r.reshape(N_TILES, P).max(axis=1)
    d1_tile = n1r.reshape(N_TILES, P).max(axis=1)
    # shared schedule across cores: slot b uses tiles 8b..8b+7
    D0 = d0_tile.reshape(NT, CORES).max(axis=1)
    D1 = d1_tile.reshape(NT, CORES).max(axis=1)
    D0 = np.maximum(D0, 1)
    D1 = np.maximum(D1, 1)

    # batches of tiles -> call schedule (identical for all cores)
    batches = [list(range(k, min(k + BATCH, NT))) for k in range(0, NT, BATCH)]
    calls = []          # (bank, [slot ids], ni, [col offset of each slot])
    stream_off = 0      # in idx elements
    for bt in batches:
        for bk, Dv in ((0, D0), (1, D1)):
            offs, c = [], 0
            for b in bt:
                offs.append(c)
                c += int(Dv[b])
            c += 1  # trailing all-pad column (trim guard)
            ni = c * P
            calls.append(dict(bank=bk, slots=bt, ni=ni, col_offs=offs,
                              cols=c, stream_off=stream_off))
            stream_off += ni
    total_idx = stream_off

    # per-edge position in its core's stream
    slot = tile_g // CORES
    core = tile_g % CORES
    batch_id = slot // BATCH
    pos_in_batch = slot % BATCH
    call_id = batch_id * 2 + bank                      # calls ordered A,B per batch
    call_off = np.array([c["stream_off"] for c in calls], np.int64)
    col_off_tab = np.zeros((len(calls), BATCH), np.int64)
    for ci, c in enumerate(calls):
        for j, o in enumerate(c["col_offs"]):
            col_off_tab[ci, j] = o

    # d = within-(node,bank) counter
    key = r * 2 + bank
    sidx = np.argsort(key, kind="stable")
    ks = key[sidx]
    change = np.r_[True, ks[1:] != ks[:-1]]
    gstart = np.where(change, np.arange(E), 0)
    gstart = np.maximum.accumulate(gstart)
    d = np.empty(E, np.int64)
    d[sidx] = np.arange(E) - gstart

    pos = call_off[call_id] + (col_off_tab[call_id, pos_in_batch] + d) * P + p
    biased = np.where(bank == 0, trow - BANK0_BASE, trow - BANK1_BASE)

    # per-core idx streams, pre-filled with per-position pad values
    pad_template = np.empty(total_idx, np.int16)
    for c in calls:
        padv = PAD0_TROW - BANK0_BASE if c["bank"] == 0 else PAD1_TROW - BANK1_BASE
        pad_template[c["stream_off"]:c["stream_off"] + c["ni"]] = padv
    idx_streams = []
    for ci in range(CORES):
        arr = pad_template.copy()
        m = core == ci
        arr[pos[m]] = biased[m].astype(np.int16)
        idx_streams.append(arr)

    # wrapped-16 layout per call, concatenated; replicated across 8 groups
    wrapped = []
    for arr in idx_streams:
        blocks = []
        for c in calls:
            v = arr[c["stream_off"]:c["stream_off"] + c["ni"]]
            blocks.append(v.reshape(-1, 16).T)         # [16, ni/16]
        w16 = np.concatenate(blocks, axis=1)           # [16, total/16]
        wrapped.append(np.tile(w16, (CORES, 1)).astype(np.int16))

    return dict(calls=calls, D0=D0, D1=D1, node_at_rank=node_at_rank,
                idx_wrapped=wrapped, total_idx=total_idx)


def _build_phase_b(calls, D0, D1):
    import concourse.bacc as bacc
    import concourse.mybir as mybir
    import concourse.tile as tile
    from concourse.masks import make_identity

    AL = mybir.AluOpType
    AF = mybir.ActivationFunctionType
    total16 = sum(c["ni"] for c in calls) // 16

    nc = bacc.Bacc("TRN2", target_bir_lowering=False, debug=False,
                   num_devices=CORES, num_swdge_queues=NQ)
    table = nc.dram_tensor("table", [TABLE_ROWS, ROW], mybir.dt.float32,
                           kind="ExternalInput")
    idxs = nc.dram_tensor("idxs", [P, total16], mybir.dt.int16,
                          kind="ExternalInput")
    adst = nc.dram_tensor("adst", [P, NT], mybir.dt.float32,
                          kind="ExternalInput")
    biasr = nc.dram_tensor("biasr", [P, HIDDEN], mybir.dt.float32,
                           kind="ExternalInput")
    linW = nc.dram_tensor("linW", [HIDDEN, OUT_CH], mybir.dt.float32,
                          kind="ExternalInput")
    linbr = nc.dram_tensor("linbr", [P, OUT_CH], mybir.dt.float32,
                           kind="ExternalInput")
    outz = nc.dram_tensor("outz", [P, NT, OUT_CH], mybir.dt.float32,
                          kind="ExternalOutput")

    bank_slice = {0: (BANK0_BASE, BANK0_BASE + 65536),
                  1: (BANK1_BASE, TABLE_ROWS)}

    with tile.TileContext(nc) as tc:
        with (
            tc.tile_pool(name="const", bufs=1) as cp,
            tc.tile_pool(name="ga", bufs=2) as gap,
            tc.tile_pool(name="gb", bufs=2) as gbp,
            tc.tile_pool(name="sc", bufs=3) as sp,
            tc.tile_pool(name="ps", bufs=2, space="PSUM") as ps,
            tc.tile_pool(name="ps2", bufs=2, space="PSUM") as ps2,
        ):
            ident = cp.tile([P, P], mybir.dt.float32)
            make_identity(nc, ident[:])
            idx_sb = cp.tile([P, total16], mybir.dt.int16)
            nc.sync.dma_start(out=idx_sb[:], in_=idxs[:, :])
            adst_sb = cp.tile([P, NT], mybir.dt.float32)
            nc.sync.dma_start(out=adst_sb[:], in_=adst[:, :])
            bias_sb = cp.tile([P, HIDDEN], mybir.dt.float32)
            nc.sync.dma_start(out=bias_sb[:], in_=biasr[:, :])
            linW_sb = cp.tile([HIDDEN, OUT_CH], mybir.dt.float32)
            nc.sync.dma_start(out=linW_sb[:], in_=linW[:, :])
            linb_sb = cp.tile([P, OUT_CH], mybir.dt.float32)
            nc.sync.dma_start(out=linb_sb[:], in_=linbr[:, :])
            ostage = cp.tile([P, NT, OUT_CH], mybir.dt.float32)

            qn = 0
            for k in range(0, len(calls), 2):
                cA, cB = calls[k], calls[k + 1]
                g = {}
                for c in (cA, cB):
                    pool = gap if c["bank"] == 0 else gbp
                    gt = pool.tile([P, c["cols"], ROW], mybir.dt.float32,
                                   tag=f"g{c['bank']}")
                    off16 = c["stream_off"] // 16
                    lo, hi = bank_slice[c["bank"]]
                    nc.gpsimd.dma_gather(
                        gt[:], table[lo:hi, :],
                        idx_sb[:, off16:off16 + c["ni"] // 16],
                        c["ni"], c["ni"], ROW,
                        single_packet=False, queue_num=qn % NQ)
                    qn += 1
                    g[c["bank"]] = gt

                for j, b in enumerate(cA["slots"]):
                    d0, d1 = int(D0[b]), int(D1[b])
                    a0 = cA["col_offs"][j]
                    a1 = cB["col_offs"][j]
                    gA, gB = g[0], g[1]
                    dt_ = d0 + d1

                    # leaky_relu(a_src + a_dst) = max(t, 0.2*t), t = a_src + a_dst
                    logit = sp.tile([P, dt_], mybir.dt.float32, tag="logit")
                    lu = sp.tile([P, dt_], mybir.dt.float32, tag="lu")
                    for gsb, aoff, dd, loff in ((gA, a0, d0, 0),
                                                (gB, a1, d1, d0)):
                        nc.vector.tensor_scalar_add(
                            out=logit[:, loff:loff + dd],
                            in0=gsb[:, aoff:aoff + dd, HIDDEN],
                            scalar1=adst_sb[:, b:b + 1])
                        nc.vector.tensor_scalar(
                            out=lu[:, loff:loff + dd],
                            in0=gsb[:, aoff:aoff + dd, HIDDEN],
                            scalar1=adst_sb[:, b:b + 1],
                            scalar2=NEG_SLOPE, op0=AL.add, op1=AL.mult)
                    nc.vector.tensor_tensor(out=logit[:], in0=logit[:],
                                            in1=lu[:], op=AL.max)
                    w = sp.tile([P, dt_], mybir.dt.float32, tag="w")
                    denom = sp.tile([P, 1], mybir.dt.float32, tag="den")
                    nc.scalar.activation(out=w[:], in_=logit[:], func=AF.Exp,
                                         accum_out=denom[:])

                    nc.vector.tensor_tensor(
                        out=gA[:, a0:a0 + d0, 0:HIDDEN],
                        in0=gA[:, a0:a0 + d0, 0:HIDDEN],
                        in1=w[:, 0:d0][:, :, None].broadcast_to([P, d0, HIDDEN]),
                        op=AL.mult)
                    nc.vector.tensor_tensor(
                        out=gB[:, a1:a1 + d1, 0:HIDDEN],
                        in0=gB[:, a1:a1 + d1, 0:HIDDEN],
                        in1=w[:, d0:dt_][:, :, None].broadcast_to([P, d1, HIDDEN]),
                        op=AL.mult)

                    agg = sp.tile([P, HIDDEN], mybir.dt.float32, tag="agg")
                    agg2 = sp.tile([P, HIDDEN], mybir.dt.float32, tag="agg2")
                    nc.vector.tensor_reduce(
                        out=agg[:],
                        in_=gA[:, a0:a0 + d0, 0:HIDDEN].rearrange("p d c -> p c d"),
                        axis=mybir.AxisListType.X, op=AL.add)
                    nc.vector.tensor_reduce(
                        out=agg2[:],
                        in_=gB[:, a1:a1 + d1, 0:HIDDEN].rearrange("p d c -> p c d"),
                        axis=mybir.AxisListType.X, op=AL.add)
                    nc.vector.tensor_tensor(out=agg[:], in0=agg[:], in1=agg2[:],
                                            op=AL.add)

                    rden = sp.tile([P, 1], mybir.dt.float32, tag="rden")
                    nc.vector.reciprocal(rden[:], denom[:])
                    nc.vector.tensor_scalar_mul(out=agg[:], in0=agg[:],
                                                scalar1=rden[:])
                    nc.vector.tensor_tensor(out=agg[:], in0=agg[:],
                                            in1=bias_sb[:], op=AL.add)
                    # ELU: elu(y) = max(y,0) + exp(min(y,0)) - 1
                    tmin = sp.tile([P, HIDDEN], mybir.dt.float32, tag="tmin")
                    nc.vector.tensor_scalar_min(out=tmin[:], in0=agg[:],
                                                scalar1=0.0)
                    nc.scalar.activation(out=tmin[:], in_=tmin[:], func=AF.Exp)
                    nc.vector.tensor_scalar_max(out=agg[:], in0=agg[:],
                                                scalar1=0.0)
                    nc.vector.tensor_scalar(out=tmin[:], in0=tmin[:],
                                            scalar1=1.0, scalar2=None,
                                            op0=AL.subtract)
                    nc.vector.tensor_tensor(out=agg[:], in0=agg[:], in1=tmin[:],
                                            op=AL.add)

                    # z = elu_out @ linW + linb, then log_softmax
                    yT_ps = ps.tile([HIDDEN, P], mybir.dt.float32, space="PSUM",
                                    tag="yT")
                    nc.tensor.transpose(out=yT_ps[:], in_=agg[:],
                                        identity=ident[:])
                    yT_sb = sp.tile([HIDDEN, P], mybir.dt.float32, tag="yT_sb")
                    nc.vector.tensor_copy(out=yT_sb[:], in_=yT_ps[:])
                    z_ps = ps2.tile([P, OUT_CH], mybir.dt.float32, space="PSUM",
                                    tag="z")
                    nc.tensor.matmul(out=z_ps[:], lhsT=yT_sb[:], rhs=linW_sb[:],
                                     start=True, stop=True)
                    zy = sp.tile([P, OUT_CH], mybir.dt.float32, tag="zy")
                    nc.vector.tensor_tensor(out=zy[:], in0=z_ps[:],
                                            in1=linb_sb[:], op=AL.add)
                    nm = sp.tile([P, 1], mybir.dt.float32, tag="nm")
                    nc.vector.tensor_reduce(out=nm[:], in_=zy[:],
                                            axis=mybir.AxisListType.X,
                                            op=AL.max)
                    nc.vector.tensor_scalar_mul(out=nm[:], in0=nm[:],
                                                scalar1=-1.0)
                    es = sp.tile([P, OUT_CH], mybir.dt.float32, tag="es")
                    s = sp.tile([P, 1], mybir.dt.float32, tag="s")
                    nc.scalar.activation(out=es[:], in_=zy[:], func=AF.Exp,
                                         bias=nm[:], accum_out=s[:])
                    ls = sp.tile([P, 1], mybir.dt.float32, tag="ls")
                    nc.scalar.activation(out=ls[:], in_=s[:], func=AF.Ln)
                    nc.vector.tensor_tensor(out=nm[:], in0=nm[:], in1=ls[:],
                                            op=AL.subtract)
                    nc.vector.tensor_scalar_add(out=ostage[:, b, :], in0=zy[:],
                                                scalar1=nm[:])
            nc.sync.dma_start(out=outz[:, :, :], in_=ostage[:])

    nc.finalize()
    return nc


EXEC_TIMES = []


def kernel(x, edge_index, W, att_src, att_dst, gat_bias, lin_W, lin_b):
    import os
    from concourse.bass_utils import run_bass_kernel_spmd
    trace = os.environ.get("GAT_TRACE") == "1"

    x = np.asarray(x, _f32)
    edge_index = np.asarray(edge_index)
    W = np.asarray(W, _f32)
    att_src = np.asarray(att_src, _f32)
    att_dst = np.asarray(att_dst, _f32)
    gat_bias = np.asarray(gat_bias, _f32)
    lin_W = np.asarray(lin_W, _f32)
    lin_b = np.asarray(lin_b, _f32)

    # ---- launch A: table rows -------------------------------------------
    nc_a = _build_phase_a()
    xT = np.ascontiguousarray(x.T)                    # [128, 100000]
    att2 = np.stack([att_src, att_dst], axis=1)       # [48, 2]
    in_maps_a = []
    for c in range(CORES):
        sh = np.zeros((P, NODES_PER_CORE), _f32)
        sh[:, :12500] = xT[:, c * 12500:(c + 1) * 12500]
        in_maps_a.append({"xT": sh, "W": W, "att": att2})
    res_a = run_bass_kernel_spmd(nc_a, in_maps_a, core_ids=list(range(CORES)), trace=trace)
    EXEC_TIMES.append(("phase_a", res_a.exec_time_ns))
    hx = np.zeros((N_NODES, ROW), _f32)
    for c in range(CORES):
        o = res_a.results[c]["hx"]                    # [128, 98, 64]
        hx[c * 12500:(c + 1) * 12500] = (
            o.transpose(1, 0, 2).reshape(NODES_PER_CORE, ROW)[:12500])

    # ---- host: edge layout ----------------------------------------------
    src = np.concatenate([edge_index[0], np.arange(N_NODES, dtype=np.int64)])
    dst = np.concatenate([edge_index[1], np.arange(N_NODES, dtype=np.int64)])
    lay = _layout(src.astype(np.int64), dst.astype(np.int64))

    table = np.zeros((TABLE_ROWS, ROW), _f32)
    table[0:PAD0_TROW] = hx[0:PAD0_TROW]
    table[PAD0_TROW, HIDDEN] = PAD_ASRC
    table[PAD0_TROW + 1:N_NODES + 1] = hx[PAD0_TROW:]
    table[PAD1_TROW, HIDDEN] = PAD_ASRC

    a_dst_vec = hx[:, HIDDEN + 1]
    node_at_rank = lay["node_at_rank"]
    adst_cores = []
    for c in range(CORES):
        arr = np.zeros((P, NT), _f32)
        for b in range(NT):
            tg = b * CORES + c
            nodes = node_at_rank[tg * P:(tg + 1) * P]
            valid = nodes >= 0
            arr[valid, b] = a_dst_vec[nodes[valid]]
        adst_cores.append(arr)

    # ---- launch B --------------------------------------------------------
    nc_b = _build_phase_b(lay["calls"], lay["D0"], lay["D1"])
    biasr = np.tile(gat_bias[None, :], (P, 1)).astype(_f32)
    linbr = np.tile(lin_b[None, :], (P, 1)).astype(_f32)
    in_maps_b = []
    for c in range(CORES):
        in_maps_b.append({
            "table": table, "idxs": lay["idx_wrapped"][c],
            "adst": adst_cores[c], "biasr": biasr,
            "linW": lin_W, "linbr": linbr,
        })
    res_b = run_bass_kernel_spmd(nc_b, in_maps_b, core_ids=list(range(CORES)), trace=trace)
    EXEC_TIMES.append(("phase_b", res_b.exec_time_ns))

    out = np.zeros((N_NODES, OUT_CH), _f32)
    for c in range(CORES):
        oz = res_b.results[c]["outz"]                 # [128, 98, 16]
        for b in range(NT):
            tg = b * CORES + c
            nodes = node_at_rank[tg * P:(tg + 1) * P]
            valid = nodes >= 0
            out[nodes[valid]] = oz[valid, b, :]
    return out
